# revision 46
# baseline (speedup 1.0000x reference)
"""LlamaPEER MoE-routing kernel for 8 NeuronCores (TRN2, Bass/Tile).

Strategy: data-parallel over B*T (2048 tokens -> 256/core), expert tables
replicated. Host concatenates e_down|e_up into one bf16 table [E, 2D] so a
single indirect-DMA gather per (token-half, slot) fetches both rows.
Per core:
  1. qT = Wq^T @ x^T on PE (fp32 -- routing is precision-critical; xT
     precomputed on host). Interleaved per head so head-0 routing starts
     after 1/4 of the qT work.
  2. sims via PE (keysT precomputed on host), top-8 via DVE max/max_index,
     K x K cross combine + second top-8, index select via iota-mask trick.
  3. Per (h,k) slot: one indirect-DMA gather of the combined bf16 row
     (token-major: partition p = token p's row); down-proj dot via one
     fused DVE tensor_tensor_reduce against resident bf16 x tile; silu on
     ACT; dg = ident * silu(dot) * relu(score) in one DVE tensor_scalar;
     up-proj diag(h) @ up-half rows on PE, accumulated in PSUM over all
     32 slots; single copy-out per 128-token half.
"""

import numpy as np
import ml_dtypes

import concourse.bass as bass
import concourse.tile as tile
from concourse import mybir
from concourse.bass_utils import run_bass_kernel_spmd
from concourse.vector_clock import ScopedClock

N_CORES = 8
B, T, D = 2, 1024, 2048
H, K, DK = 4, 8, 64
E = 16384
NK = 128
TOK_PER_CORE = (B * T) // N_CORES  # 256
NHALF = TOK_PER_CORE // 128  # 2
NSLOT = H * K  # 32
FP = mybir.dt.float32
BF = mybir.dt.bfloat16
I32 = mybir.dt.int32
U32 = mybir.dt.uint32

# --- workaround: this walrus build allows only 1 sync-wait command on the
# final SP drain; split the tile-context drain into 1-wait drains.
_MAX_DRAIN_WAITS = 1


def _patched_drain_and_barrier(self, tick_clock, wait_clock):
    nc = self.nc
    drain_inst = nc.sync.drain()
    wait_clock.add_sem_waits(
        drain_inst.ins, ScopedClock({None: tick_clock.global_clock})
    )
    si = drain_inst.ins.sync_info
    if si is not None and len(si.on_wait) > _MAX_DRAIN_WAITS:
        waits = list(si.on_wait)
        upds = list(si.on_update)
        drain_inst.ins.sync_info = mybir.SyncInfo(
            on_wait=waits[:_MAX_DRAIN_WAITS], on_update=[]
        )
        rest = waits[_MAX_DRAIN_WAITS:]
        while rest:
            extra = nc.sync.drain()
            extra.ins.sync_info = mybir.SyncInfo(
                on_wait=rest[:_MAX_DRAIN_WAITS],
                on_update=upds if len(rest) <= _MAX_DRAIN_WAITS else [],
            )
            rest = rest[_MAX_DRAIN_WAITS:]
    nc.all_engine_barrier()
    popped = nc._tile_sem_poison_stack.pop()
    assert popped is self._sem_poison
    all_sems = list(self.sems.allocated().values())
    for i in range(0, len(all_sems), 8):
        nc.clear_and_free_semaphores(all_sems[i : i + 8])
    nc.all_engine_barrier()


tile.TileContext._drain_and_barrier = _patched_drain_and_barrier

_orig_lower_ordered = tile.TileContext._lower_ordered_insts


def _patched_lower_ordered(self, postordered_blocks):
    # this walrus build supports only one sync-wait command per instruction:
    # hoist extra waits onto same-engine NoOps placed just before.
    for bb_name, insts in postordered_blocks.items():
        new = []
        for inst in insts:
            si = getattr(inst, "sync_info", None)
            eng = getattr(inst, "engine", None)
            if si is not None and eng is not None and len(si.on_wait) > 1:
                waits = list(si.on_wait)
                for w in waits[:-1]:
                    nop = mybir.InstNoOp(
                        name=self.nc.get_next_instruction_name(),
                        sync_info=mybir.SyncInfo(on_wait=[w], on_update=[]),
                        bass_nofuse=True,
                        engine=eng,
                    )
                    new.append(nop)
                inst.sync_info = mybir.SyncInfo(
                    on_wait=[waits[-1]], on_update=list(si.on_update)
                )
            new.append(inst)
        insts[:] = new
    return _orig_lower_ordered(self, postordered_blocks)


tile.TileContext._lower_ordered_insts = _patched_lower_ordered


def _re(ap, dims):
    """Return ap with its free-axis access pattern replaced by `dims`
    (list of [step, count]); keeps the partition dim."""
    return ap.__replace__(ap=[list(ap.ap)[0]] + [list(d) for d in dims])


def build_program():
    nc = bass.Bass("TRN2", target_bir_lowering=False, debug=False)

    # host pre-laid-out for contiguous per-partition loads:
    # xbf[p, hf*D+d], xt[p, c*256+t], wq[m*128+p, c*128+o]
    xbf_d = nc.dram_tensor("xbf", [128, NHALF * D], BF, kind="ExternalInput")
    xt_d = nc.dram_tensor("xt", [128, (D // 128) * TOK_PER_CORE], FP, kind="ExternalInput")
    wq_d = nc.dram_tensor("wq", [4 * 128, (D // 128) * 128], FP, kind="ExternalInput")
    kt_d = nc.dram_tensor("keyst", [2 * DK, H * NK], FP, kind="ExternalInput")
    cat_d = nc.dram_tensor("ecat", [E, 2 * D], BF, kind="ExternalInput")
    id_d = nc.dram_tensor("ident", [128, 128], BF, kind="ExternalInput")
    io_d = nc.dram_tensor("iota64", [128, 64], FP, kind="ExternalInput")
    out_d = nc.dram_tensor("out", [TOK_PER_CORE, D], FP, kind="ExternalOutput")

    NDCH = D // 128  # 16 d-chunks

    with tile.TileContext(nc) as tc:
        with (
            tc.tile_pool(name="const", bufs=1) as cpool,
            tc.tile_pool(name="mats", bufs=1) as mpool,
            tc.tile_pool(name="route", bufs=4) as rpool,
            tc.tile_pool(name="persist", bufs=1) as ppool,
            tc.tile_pool(name="gd", bufs=11) as gdpool,
            tc.tile_pool(name="scr", bufs=2) as spool,
            tc.tile_pool(name="dg", bufs=10) as dgpool,
            tc.tile_pool(name="ob", bufs=1) as opool,
            tc.tile_pool(name="psq", bufs=4, space="PSUM") as psq,
            tc.tile_pool(name="psacc", bufs=1, space="PSUM") as psacc,
        ):
            # PE warm-up: the HAM throttle needs ~3.4us of continuous
            # activity before the PE runs at full clock. While the input
            # DMAs stream (~17us), keep the PE busy with dummy matmuls on
            # a memset tile so qT starts warm instead of at half clock.
            warm = cpool.tile([128, 256], BF, tag="warm", name="warm")
            nc.vector.memset(warm[:], 0.0)
            pwarm = psq.tile([128, 256], FP, tag="psq")
            for _ in range(60):
                nc.tensor.matmul(
                    pwarm[:],
                    lhsT=warm[:, 0:128],
                    rhs=warm[:],
                    start=True,
                    stop=True,
                )

            # load order tuned for pipeline startup: xt + wq m-group 0
            # first (qT m0 can start ~10us in), tiny/late-needed after.
            # all loads are contiguous per partition (host pre-laid-out).
            half_cols = (NDCH // 2) * TOK_PER_CORE
            xt_a = mpool.tile([128, half_cols], FP, tag="xt_a", name="xt_a")
            xt_b = mpool.tile([128, half_cols], FP, tag="xt_b", name="xt_b")

            def xt_chunk(c):
                # qT chunk c's rhs: chunks 0-7 live in xt_a, 8-15 in xt_b
                t = xt_a if c < NDCH // 2 else xt_b
                c2 = c % (NDCH // 2)
                return t[:, c2 * TOK_PER_CORE : (c2 + 1) * TOK_PER_CORE]

            nc.sync.dma_start(xt_a[:], xt_d.ap()[:, :half_cols])
            wqm = []
            for m in range(4):
                wt = mpool.tile([128, NDCH * 128], FP, tag=f"wqm{m}", name=f"wqm{m}")
                nc.sync.dma_start(wt[:], wq_d.ap()[m * 128 : (m + 1) * 128, :])
                wqm.append(wt)
                if m == 0:
                    nc.sync.dma_start(xt_b[:], xt_d.ap()[:, half_cols:])
                    kt_sb = mpool.tile([2 * DK, H * NK], FP)
                    nc.sync.dma_start(kt_sb[:], kt_d.ap())
                    # x rows in bf16 for the down-proj dot (both halves)
                    x_bf = mpool.tile([128, NHALF * D], BF)
                    nc.sync.dma_start(x_bf[:], xbf_d.ap())
                    ident = cpool.tile([128, 128], BF)
                    nc.sync.dma_start(ident[:], id_d.ap())
                    iota = cpool.tile([128, 64], FP)
                    nc.sync.dma_start(iota[:], io_d.ap())

            qt_sb = ppool.tile([128, 4 * TOK_PER_CORE], FP)
            fi_all = [
                ppool.tile([128, NSLOT], I32, tag=f"fi{hf}", name=f"fi{hf}")
                for hf in range(NHALF)
            ]
            fsr_all = [
                ppool.tile([128, NSLOT], FP, tag=f"fsr{hf}", name=f"fsr{hf}")
                for hf in range(NHALF)
            ]
            hid_all = [
                ppool.tile([128, NSLOT], FP, tag=f"hid{hf}", name=f"hid{hf}")
                for hf in range(NHALF)
            ]
            hsil_all = [
                ppool.tile([128, NSLOT], FP, tag=f"hsil{hf}", name=f"hsil{hf}")
                for hf in range(NHALF)
            ]

            # ---- phase 1: qT + routing. PE emission order is
            # m0, m1, rt0, m2, rt1, m3, rt2, rt3 so the PE never waits on
            # the PSUM->SBUF copy of the m-group it needs for sims (the
            # copy runs during the next m-group's matmuls), keeping the
            # tensor engine continuously busy so its p-state ramps. ----
            def emit_qt(h):
                pq = psq.tile([128, TOK_PER_CORE], FP, tag="psq")
                for c in range(NDCH):
                    nc.tensor.matmul(
                        pq[:],
                        lhsT=wqm[h][:, c * 128 : (c + 1) * 128],
                        rhs=xt_chunk(c),
                        start=(c == 0),
                        stop=(c == NDCH - 1),
                    )
                nc.vector.tensor_copy(
                    qt_sb[:, h * TOK_PER_CORE : (h + 1) * TOK_PER_CORE], pq[:]
                )

            def emit_routing(h):
                for hf in range(NHALF):
                    t0 = hf * 128
                    ss = []
                    ii = []
                    for sub in range(2):
                        ps = psq.tile([128, NK], FP, tag="psq")
                        nc.tensor.matmul(
                            ps[:],
                            lhsT=qt_sb[
                                sub * 64 : (sub + 1) * 64,
                                h * TOK_PER_CORE + t0 : h * TOK_PER_CORE + t0 + 128,
                            ],
                            rhs=kt_sb[sub * 64 : (sub + 1) * 64, h * NK : (h + 1) * NK],
                            start=True,
                            stop=True,
                        )
                        sim = rpool.tile([128, NK], FP, tag="sim")
                        nc.vector.tensor_copy(sim[:], ps[:])
                        s = rpool.tile([128, 8], FP, tag="s")
                        nc.vector.max(s[:], sim[:])
                        idx = rpool.tile([128, 8], U32, tag="idx")
                        nc.vector.max_index(idx[:], s[:], sim[:])
                        idf = rpool.tile([128, 8], FP, tag="idf")
                        nc.vector.tensor_copy(idf[:], idx[:])
                        ss.append(s)
                        ii.append(idf)
                    # cross combine: [128, 8(k1), 8(k2)]
                    alls = rpool.tile([128, 64], FP, tag="alls")
                    a3 = _re(alls[:], [[8, 8], [1, 8]])
                    nc.vector.tensor_tensor(
                        out=a3,
                        in0=_re(ss[0][:], [[1, 8], [0, 8]]),
                        in1=_re(ss[1][:], [[0, 8], [1, 8]]),
                        op=mybir.AluOpType.add,
                    )
                    alli = rpool.tile([128, 64], FP, tag="alli")
                    ai3 = _re(alli[:], [[8, 8], [1, 8]])
                    nc.vector.tensor_scalar(
                        out=ai3,
                        in0=_re(ii[0][:], [[1, 8], [0, 8]]),
                        scalar1=float(NK),
                        scalar2=None,
                        op0=mybir.AluOpType.mult,
                    )
                    nc.vector.tensor_tensor(
                        out=ai3,
                        in0=ai3,
                        in1=_re(ii[1][:], [[0, 8], [1, 8]]),
                        op=mybir.AluOpType.add,
                    )
                    fs = rpool.tile([128, 8], FP, tag="fs")
                    nc.vector.max(fs[:], alls[:])
                    pk = rpool.tile([128, 8], U32, tag="pk")
                    nc.vector.max_index(pk[:], fs[:], alls[:])
                    pkf = rpool.tile([128, 8], FP, tag="pkf")
                    nc.vector.tensor_copy(pkf[:], pk[:])
                    # mask[p, j, n] = (pk[p,j] == iota[p,n])
                    mask = rpool.tile([128, 512], FP, tag="mask")
                    m3 = _re(mask[:], [[64, 8], [1, 64]])
                    nc.vector.tensor_tensor(
                        out=m3,
                        in0=_re(pkf[:], [[1, 8], [0, 64]]),
                        in1=_re(iota[:], [[0, 8], [1, 64]]),
                        op=mybir.AluOpType.is_equal,
                    )
                    nc.vector.tensor_tensor(
                        out=m3,
                        in0=m3,
                        in1=_re(alli[:], [[0, 8], [1, 64]]),
                        op=mybir.AluOpType.mult,
                    )
                    fif = rpool.tile([128, 8], FP, tag="fif")
                    nc.vector.tensor_reduce(
                        fif[:],
                        m3,
                        axis=mybir.AxisListType.X,
                        op=mybir.AluOpType.add,
                    )
                    nc.vector.tensor_copy(fi_all[hf][:, h * 8 : (h + 1) * 8], fif[:])
                    nc.vector.tensor_scalar_max(
                        fsr_all[hf][:, h * 8 : (h + 1) * 8], fs[:], 0.0
                    )

            emit_qt(0)
            emit_routing(0)
            emit_qt(1)
            emit_routing(1)
            emit_qt(2)
            emit_routing(2)
            emit_qt(3)
            emit_routing(3)

            # ---- phase 2: per-slot gather + fused down/up projection.
            # Slots processed in groups of GRP: gathers/dots stream in slot
            # order, but the up-proj matmuls are emitted in REVERSE slot
            # order within each group so the PE idles once per group and
            # then bursts 4*GRP matmuls back-to-back (keeps the HAM
            # throttle/p-state at full clock instead of oscillating
            # per-slot). silu is batched per group. ----
            GROUPS = [list(range(g, g + 4)) for g in range(0, 28, 4)] + [
                [28, 29],
                [30, 31],
            ]
            # In the second half the PE has built up a queue backlog
            # (prologue qT/sims overlaps the first half's stream), so it
            # trails the gather stream at the tail. Offload the up-proj of
            # these hf1 slots to the DVE (scale + add into an SBUF
            # accumulator, merged with the PSUM accumulator at the end).
            MOVED = {0: set(), 1: {3, 7, 11, 15, 19, 23, 27}}
            acc_sb = ppool.tile([128, D], FP, tag="acc_sb", name="acc_sb")
            for hf in range(NHALF):
                t0 = hf * 128
                moved = MOVED[hf]
                pe_grps = [
                    (gi, [k for k in grp if k not in moved])
                    for gi, grp in enumerate(GROUPS)
                ]
                pe_first = next(g[1][-1] for g in pe_grps if g[1])
                pe_last_gi, pe_last_grp = [g for g in pe_grps if g[1]][-1]
                first_moved = min(moved) if moved else None
                acc = psacc.tile([128, D], FP, tag="acc")
                cts = {}
                for gi, grp in enumerate(GROUPS):
                    for k in grp:
                        ct = gdpool.tile([128, 2 * D], BF, tag="ct")
                        cts[k] = ct
                        nc.gpsimd.indirect_dma_start(
                            out=ct[:],
                            out_offset=None,
                            in_=cat_d.ap(),
                            in_offset=bass.IndirectOffsetOnAxis(
                                ap=fi_all[hf][:, k : k + 1], axis=0
                            ),
                        )
                        # down-proj dot: hid[:,k] = sum(x_bf * ct[:, :D])
                        scr = spool.tile([128, D], BF, tag="scr")
                        nc.vector.tensor_tensor(
                            out=scr[:],
                            in0=ct[:, 0:D],
                            in1=x_bf[:, hf * D : (hf + 1) * D],
                            op=mybir.AluOpType.mult,
                        )
                        scr2 = spool.tile([128, D], BF, tag="scr2")
                        nc.scalar.activation(
                            scr2[:],
                            scr[:],
                            mybir.ActivationFunctionType.Copy,
                            accum_out=hid_all[hf][:, k : k + 1],
                        )
                    # h = silu(dot) for the whole group in one ACT op
                    nc.scalar.activation(
                        hsil_all[hf][:, grp[0] : grp[-1] + 1],
                        hid_all[hf][:, grp[0] : grp[-1] + 1],
                        mybir.ActivationFunctionType.Silu,
                    )
                    dgs = {}
                    for k in grp:
                        if k in moved:
                            # DVE up-proj: acc_sb (+)= ct_up * silu(dot) * relu(s)
                            if k == first_moved:
                                nc.vector.tensor_scalar(
                                    out=acc_sb[:],
                                    in0=cts[k][:, D : 2 * D],
                                    scalar1=hsil_all[hf][:, k : k + 1],
                                    scalar2=fsr_all[hf][:, k : k + 1],
                                    op0=mybir.AluOpType.mult,
                                    op1=mybir.AluOpType.mult,
                                )
                            else:
                                tmp = spool.tile([128, D], BF, tag="utmp")
                                nc.vector.tensor_scalar(
                                    out=tmp[:],
                                    in0=cts[k][:, D : 2 * D],
                                    scalar1=hsil_all[hf][:, k : k + 1],
                                    scalar2=fsr_all[hf][:, k : k + 1],
                                    op0=mybir.AluOpType.mult,
                                    op1=mybir.AluOpType.mult,
                                )
                                nc.vector.tensor_tensor(
                                    out=acc_sb[:],
                                    in0=acc_sb[:],
                                    in1=tmp[:],
                                    op=mybir.AluOpType.add,
                                )
                            continue
                        dg = dgpool.tile([128, 128], BF, tag="dg")
                        dgs[k] = dg
                        nc.vector.tensor_scalar(
                            out=dg[:],
                            in0=ident[:],
                            scalar1=hsil_all[hf][:, k : k + 1],
                            scalar2=fsr_all[hf][:, k : k + 1],
                            op0=mybir.AluOpType.mult,
                            op1=mybir.AluOpType.mult,
                        )
                    for k in reversed(grp):
                        if k in moved:
                            continue
                        for c in range(4):
                            nc.tensor.matmul(
                                acc[:, c * 512 : (c + 1) * 512],
                                lhsT=dgs[k][:],
                                rhs=cts[k][:, D + c * 512 : D + (c + 1) * 512],
                                start=(gi == 0 and k == pe_first),
                                stop=(gi == pe_last_gi and k == pe_last_grp[0]),
                            )
                ob = opool.tile([128, D], FP, tag="ob")
                if moved:
                    nc.vector.tensor_tensor(
                        out=ob[:], in0=acc[:], in1=acc_sb[:], op=mybir.AluOpType.add
                    )
                else:
                    nc.vector.tensor_copy(ob[:], acc[:])
                nc.sync.dma_start(out_d.ap()[t0 : t0 + 128, :], ob[:])

    return nc


_CACHED = {}


def kernel(x, Wq, keys, e_down, e_up):
    x = np.asarray(x, dtype=np.float32)
    Wq = np.asarray(Wq, dtype=np.float32)
    keys = np.asarray(keys, dtype=np.float32)
    e_down = np.asarray(e_down, dtype=np.float32)
    e_up = np.asarray(e_up, dtype=np.float32)

    if "nc" not in _CACHED:
        _CACHED["nc"] = build_program()
    nc = _CACHED["nc"]

    xf = x.reshape(B * T, D)
    keyst = np.ascontiguousarray(keys.transpose(2, 3, 0, 1)).reshape(2 * DK, H * NK)
    # keyst[sub*64+dk, h*NK + nk] = keys[h, nk, sub, dk]
    ident = np.eye(128, dtype=ml_dtypes.bfloat16)
    iota64 = np.tile(np.arange(64, dtype=np.float32), (128, 1))

    # combined bf16 expert table: row e = e_down[e] || e_up[e]
    ecat = np.empty((E, 2 * D), dtype=ml_dtypes.bfloat16)
    ecat[:, :D] = e_down.astype(ml_dtypes.bfloat16)
    ecat[:, D:] = e_up.astype(ml_dtypes.bfloat16)

    xbf = xf.astype(ml_dtypes.bfloat16)
    NDCH = D // 128
    # wq_r[m*128+p, c*128+o] = Wq[c*128+p, m*128+o]
    wq_r = np.ascontiguousarray(
        Wq.reshape(NDCH, 128, 4, 128).transpose(2, 1, 0, 3).reshape(512, NDCH * 128)
    )

    in_maps = []
    for c in range(N_CORES):
        xs = np.ascontiguousarray(xf[c * TOK_PER_CORE : (c + 1) * TOK_PER_CORE])
        xbf_c = xbf[c * TOK_PER_CORE : (c + 1) * TOK_PER_CORE]
        # xbf_r[p, hf*D+d] = xbf_c[hf*128+p, d]
        xbf_r = np.ascontiguousarray(
            xbf_c.reshape(NHALF, 128, D).transpose(1, 0, 2).reshape(128, NHALF * D)
        )
        # xt_r[p, ch*256+t] = xs[t, ch*128+p]
        xt_r = np.ascontiguousarray(
            xs.T.reshape(NDCH, 128, TOK_PER_CORE)
            .transpose(1, 0, 2)
            .reshape(128, NDCH * TOK_PER_CORE)
        )
        in_maps.append(
            {
                "xbf": xbf_r,
                "xt": xt_r,
                "wq": wq_r,
                "keyst": keyst,
                "ecat": ecat,
                "ident": ident,
                "iota64": iota64,
            }
        )

    res = run_bass_kernel_spmd(nc, in_maps, core_ids=list(range(N_CORES)))
    _CACHED["res"] = res
    out = np.concatenate([res.results[c]["out"] for c in range(N_CORES)], axis=0)
    return out.reshape(B, T, D)


# revision 52
# speedup vs baseline: 1.0136x; 1.0136x over previous
"""LlamaPEER MoE-routing kernel for 8 NeuronCores (TRN2, Bass/Tile).

Strategy: data-parallel over B*T (2048 tokens -> 256/core), expert tables
replicated. Host concatenates e_down|e_up into one bf16 table [E, 2D] so a
single indirect-DMA gather per (token-half, slot) fetches both rows.
Per core:
  1. qT = Wq^T @ x^T on PE (fp32 -- routing is precision-critical; xT
     precomputed on host). Interleaved per head so head-0 routing starts
     after 1/4 of the qT work.
  2. sims via PE (keysT precomputed on host), top-8 via DVE max/max_index,
     K x K cross combine + second top-8, index select via iota-mask trick.
  3. Per (h,k) slot: one indirect-DMA gather of the combined bf16 row
     (token-major: partition p = token p's row); down-proj dot via one
     fused DVE tensor_tensor_reduce against resident bf16 x tile; silu on
     ACT; dg = ident * silu(dot) * relu(score) in one DVE tensor_scalar;
     up-proj diag(h) @ up-half rows on PE, accumulated in PSUM over all
     32 slots; single copy-out per 128-token half.
"""

import numpy as np
import ml_dtypes

import concourse.bass as bass
import concourse.tile as tile
from concourse import mybir
from concourse.bass_utils import run_bass_kernel_spmd
from concourse.vector_clock import ScopedClock

N_CORES = 8
B, T, D = 2, 1024, 2048
H, K, DK = 4, 8, 64
E = 16384
NK = 128
TOK_PER_CORE = (B * T) // N_CORES  # 256
NHALF = TOK_PER_CORE // 128  # 2
NSLOT = H * K  # 32
FP = mybir.dt.float32
BF = mybir.dt.bfloat16
I32 = mybir.dt.int32
U32 = mybir.dt.uint32

# --- workaround: this walrus build allows only 1 sync-wait command on the
# final SP drain; split the tile-context drain into 1-wait drains.
_MAX_DRAIN_WAITS = 1


def _patched_drain_and_barrier(self, tick_clock, wait_clock):
    nc = self.nc
    drain_inst = nc.sync.drain()
    wait_clock.add_sem_waits(
        drain_inst.ins, ScopedClock({None: tick_clock.global_clock})
    )
    si = drain_inst.ins.sync_info
    if si is not None and len(si.on_wait) > _MAX_DRAIN_WAITS:
        waits = list(si.on_wait)
        upds = list(si.on_update)
        drain_inst.ins.sync_info = mybir.SyncInfo(
            on_wait=waits[:_MAX_DRAIN_WAITS], on_update=[]
        )
        rest = waits[_MAX_DRAIN_WAITS:]
        while rest:
            extra = nc.sync.drain()
            extra.ins.sync_info = mybir.SyncInfo(
                on_wait=rest[:_MAX_DRAIN_WAITS],
                on_update=upds if len(rest) <= _MAX_DRAIN_WAITS else [],
            )
            rest = rest[_MAX_DRAIN_WAITS:]
    nc.all_engine_barrier()
    popped = nc._tile_sem_poison_stack.pop()
    assert popped is self._sem_poison
    all_sems = list(self.sems.allocated().values())
    for i in range(0, len(all_sems), 16):
        nc.clear_and_free_semaphores(all_sems[i : i + 16])
    nc.all_engine_barrier()


tile.TileContext._drain_and_barrier = _patched_drain_and_barrier

_orig_lower_ordered = tile.TileContext._lower_ordered_insts


def _patched_lower_ordered(self, postordered_blocks):
    # this walrus build supports only one sync-wait command per instruction:
    # hoist extra waits onto same-engine NoOps placed just before.
    for bb_name, insts in postordered_blocks.items():
        new = []
        for inst in insts:
            si = getattr(inst, "sync_info", None)
            eng = getattr(inst, "engine", None)
            if si is not None and eng is not None and len(si.on_wait) > 1:
                waits = list(si.on_wait)
                for w in waits[:-1]:
                    nop = mybir.InstNoOp(
                        name=self.nc.get_next_instruction_name(),
                        sync_info=mybir.SyncInfo(on_wait=[w], on_update=[]),
                        bass_nofuse=True,
                        engine=eng,
                    )
                    new.append(nop)
                inst.sync_info = mybir.SyncInfo(
                    on_wait=[waits[-1]], on_update=list(si.on_update)
                )
            new.append(inst)
        insts[:] = new
    return _orig_lower_ordered(self, postordered_blocks)


tile.TileContext._lower_ordered_insts = _patched_lower_ordered


def _re(ap, dims):
    """Return ap with its free-axis access pattern replaced by `dims`
    (list of [step, count]); keeps the partition dim."""
    return ap.__replace__(ap=[list(ap.ap)[0]] + [list(d) for d in dims])


def build_program():
    nc = bass.Bass("TRN2", target_bir_lowering=False, debug=False)

    # host pre-laid-out for contiguous per-partition loads:
    # xbf[p, hf*D+d], xt[p, c*256+t], wq[m*128+p, c*128+o]
    xbf_d = nc.dram_tensor("xbf", [128, NHALF * D], BF, kind="ExternalInput")
    xt_d = nc.dram_tensor("xt", [128, (D // 128) * TOK_PER_CORE], FP, kind="ExternalInput")
    wq_d = nc.dram_tensor("wq", [4 * 128, (D // 128) * 128], FP, kind="ExternalInput")
    kt_d = nc.dram_tensor("keyst", [2 * DK, H * NK], FP, kind="ExternalInput")
    cat_d = nc.dram_tensor("ecat", [E, 2 * D], BF, kind="ExternalInput")
    id_d = nc.dram_tensor("ident", [128, 128], BF, kind="ExternalInput")
    io_d = nc.dram_tensor("iota64", [128, 64], FP, kind="ExternalInput")
    out_d = nc.dram_tensor("out", [TOK_PER_CORE, D], FP, kind="ExternalOutput")

    NDCH = D // 128  # 16 d-chunks

    with tile.TileContext(nc) as tc:
        with (
            tc.tile_pool(name="const", bufs=1) as cpool,
            tc.tile_pool(name="mats", bufs=1) as mpool,
            tc.tile_pool(name="route", bufs=4) as rpool,
            tc.tile_pool(name="persist", bufs=1) as ppool,
            tc.tile_pool(name="gd", bufs=11) as gdpool,
            tc.tile_pool(name="scr", bufs=2) as spool,
            tc.tile_pool(name="dg", bufs=10) as dgpool,
            tc.tile_pool(name="ob", bufs=1) as opool,
            tc.tile_pool(name="psq", bufs=4, space="PSUM") as psq,
            tc.tile_pool(name="psacc", bufs=1, space="PSUM") as psacc,
        ):
            # PE warm-up: the HAM throttle needs ~3.4us of continuous
            # activity before the PE runs at full clock. While the input
            # DMAs stream (~17us), keep the PE busy with dummy matmuls on
            # a memset tile so qT starts warm instead of at half clock.
            warm = cpool.tile([128, 256], BF, tag="warm", name="warm")
            nc.vector.memset(warm[:], 0.0)
            pwarm = psq.tile([128, 256], FP, tag="psq")
            for _ in range(60):
                nc.tensor.matmul(
                    pwarm[:],
                    lhsT=warm[:, 0:128],
                    rhs=warm[:],
                    start=True,
                    stop=True,
                )

            # load order tuned for pipeline startup: xt + wq m-group 0
            # first (qT m0 can start ~10us in), tiny/late-needed after.
            # all loads are contiguous per partition (host pre-laid-out).
            half_cols = (NDCH // 2) * TOK_PER_CORE
            xt_a = mpool.tile([128, half_cols], FP, tag="xt_a", name="xt_a")
            xt_b = mpool.tile([128, half_cols], FP, tag="xt_b", name="xt_b")

            def xt_chunk(c, a=0, b=TOK_PER_CORE):
                # qT chunk c's rhs: chunks 0-7 live in xt_a, 8-15 in xt_b
                t = xt_a if c < NDCH // 2 else xt_b
                c2 = c % (NDCH // 2)
                return t[:, c2 * TOK_PER_CORE + a : c2 * TOK_PER_CORE + b]

            nc.sync.dma_start(xt_a[:], xt_d.ap()[:, :half_cols])
            wqm = []
            for m in range(4):
                wt = mpool.tile([128, NDCH * 128], FP, tag=f"wqm{m}", name=f"wqm{m}")
                nc.sync.dma_start(wt[:], wq_d.ap()[m * 128 : (m + 1) * 128, :])
                wqm.append(wt)
                if m == 0:
                    nc.sync.dma_start(xt_b[:], xt_d.ap()[:, half_cols:])
                    kt_sb = mpool.tile([2 * DK, H * NK], FP)
                    nc.sync.dma_start(kt_sb[:], kt_d.ap())
                    # x rows in bf16 for the down-proj dot (both halves)
                    x_bf = mpool.tile([128, NHALF * D], BF)
                    nc.sync.dma_start(x_bf[:], xbf_d.ap())
                    ident = cpool.tile([128, 128], BF)
                    nc.sync.dma_start(ident[:], id_d.ap())
                    iota = cpool.tile([128, 64], FP)
                    nc.sync.dma_start(iota[:], io_d.ap())

            qt_sb = ppool.tile([128, 4 * TOK_PER_CORE], FP)
            fi_all = [
                ppool.tile([128, NSLOT], I32, tag=f"fi{hf}", name=f"fi{hf}")
                for hf in range(NHALF)
            ]
            fsr_all = [
                ppool.tile([128, NSLOT], FP, tag=f"fsr{hf}", name=f"fsr{hf}")
                for hf in range(NHALF)
            ]
            hid_all = [
                ppool.tile([128, NSLOT], FP, tag=f"hid{hf}", name=f"hid{hf}")
                for hf in range(NHALF)
            ]
            hsil_all = [
                ppool.tile([128, NSLOT], FP, tag=f"hsil{hf}", name=f"hsil{hf}")
                for hf in range(NHALF)
            ]

            # ---- phase 1: qT + routing. PE emission order is
            # m0, m1, rt0, m2, rt1, m3, rt2, rt3 so the PE never waits on
            # the PSUM->SBUF copy of the m-group it needs for sims (the
            # copy runs during the next m-group's matmuls), keeping the
            # tensor engine continuously busy so its p-state ramps. ----
            def emit_qt(h, tr=(0, TOK_PER_CORE)):
                a, b = tr
                pq = psq.tile([128, b - a], FP, tag="psq")
                for c in range(NDCH):
                    nc.tensor.matmul(
                        pq[:],
                        lhsT=wqm[h][:, c * 128 : (c + 1) * 128],
                        rhs=xt_chunk(c, a, b),
                        start=(c == 0),
                        stop=(c == NDCH - 1),
                    )
                nc.vector.tensor_copy(
                    qt_sb[:, h * TOK_PER_CORE + a : h * TOK_PER_CORE + b], pq[:]
                )

            def emit_routing(h, halves=(0, 1)):
                for hf in halves:
                    t0 = hf * 128
                    ss = []
                    ii = []
                    for sub in range(2):
                        ps = psq.tile([128, NK], FP, tag="psq")
                        nc.tensor.matmul(
                            ps[:],
                            lhsT=qt_sb[
                                sub * 64 : (sub + 1) * 64,
                                h * TOK_PER_CORE + t0 : h * TOK_PER_CORE + t0 + 128,
                            ],
                            rhs=kt_sb[sub * 64 : (sub + 1) * 64, h * NK : (h + 1) * NK],
                            start=True,
                            stop=True,
                        )
                        s = rpool.tile([128, 8], FP, tag="s")
                        nc.vector.max(s[:], ps[:])
                        idx = rpool.tile([128, 8], U32, tag="idx")
                        nc.vector.max_index(idx[:], s[:], ps[:])
                        idf = rpool.tile([128, 8], FP, tag="idf")
                        nc.vector.tensor_copy(idf[:], idx[:])
                        ss.append(s)
                        ii.append(idf)
                    # cross combine: [128, 8(k1), 8(k2)]
                    alls = rpool.tile([128, 64], FP, tag="alls")
                    a3 = _re(alls[:], [[8, 8], [1, 8]])
                    nc.vector.tensor_tensor(
                        out=a3,
                        in0=_re(ss[0][:], [[1, 8], [0, 8]]),
                        in1=_re(ss[1][:], [[0, 8], [1, 8]]),
                        op=mybir.AluOpType.add,
                    )
                    alli = rpool.tile([128, 64], FP, tag="alli")
                    ai3 = _re(alli[:], [[8, 8], [1, 8]])
                    nc.vector.tensor_scalar(
                        out=ai3,
                        in0=_re(ii[0][:], [[1, 8], [0, 8]]),
                        scalar1=float(NK),
                        scalar2=None,
                        op0=mybir.AluOpType.mult,
                    )
                    nc.vector.tensor_tensor(
                        out=ai3,
                        in0=ai3,
                        in1=_re(ii[1][:], [[0, 8], [1, 8]]),
                        op=mybir.AluOpType.add,
                    )
                    fs = rpool.tile([128, 8], FP, tag="fs")
                    nc.vector.max(fs[:], alls[:])
                    pk = rpool.tile([128, 8], U32, tag="pk")
                    nc.vector.max_index(pk[:], fs[:], alls[:])
                    pkf = rpool.tile([128, 8], FP, tag="pkf")
                    nc.vector.tensor_copy(pkf[:], pk[:])
                    # mask[p, j, n] = (pk[p,j] == iota[p,n])
                    mask = rpool.tile([128, 512], FP, tag="mask")
                    m3 = _re(mask[:], [[64, 8], [1, 64]])
                    nc.vector.tensor_tensor(
                        out=m3,
                        in0=_re(pkf[:], [[1, 8], [0, 64]]),
                        in1=_re(iota[:], [[0, 8], [1, 64]]),
                        op=mybir.AluOpType.is_equal,
                    )
                    nc.vector.tensor_tensor(
                        out=m3,
                        in0=m3,
                        in1=_re(alli[:], [[0, 8], [1, 64]]),
                        op=mybir.AluOpType.mult,
                    )
                    fif = rpool.tile([128, 8], FP, tag="fif")
                    nc.vector.tensor_reduce(
                        fif[:],
                        m3,
                        axis=mybir.AxisListType.X,
                        op=mybir.AluOpType.add,
                    )
                    nc.vector.tensor_copy(fi_all[hf][:, h * 8 : (h + 1) * 8], fif[:])
                    nc.vector.tensor_scalar_max(
                        fsr_all[hf][:, h * 8 : (h + 1) * 8], fs[:], 0.0
                    )

            # head 0's qT is split by token-half so the first half's
            # routing (and therefore the first gathers) starts after only
            # 16 half-width matmuls instead of the full m-group.
            emit_qt(0, tr=(0, 128))
            emit_routing(0, halves=(0,))
            emit_qt(0, tr=(128, 256))
            emit_routing(0, halves=(1,))
            emit_qt(1)
            emit_routing(1)
            emit_qt(2)
            emit_routing(2)
            emit_qt(3)
            emit_routing(3)

            # ---- phase 2: per-slot gather + fused down/up projection.
            # Slots processed in groups of GRP: gathers/dots stream in slot
            # order, but the up-proj matmuls are emitted in REVERSE slot
            # order within each group so the PE idles once per group and
            # then bursts 4*GRP matmuls back-to-back (keeps the HAM
            # throttle/p-state at full clock instead of oscillating
            # per-slot). silu is batched per group. ----
            GROUPS = [list(range(g, g + 4)) for g in range(0, 28, 4)] + [
                [28, 29],
                [30, 31],
            ]
            # In the second half the PE has built up a queue backlog
            # (prologue qT/sims overlaps the first half's stream), so it
            # trails the gather stream at the tail. Offload the up-proj of
            # these hf1 slots to the DVE (scale + add into an SBUF
            # accumulator, merged with the PSUM accumulator at the end).
            MOVED = {0: set(), 1: {3, 7, 11, 15, 19, 23, 27}}
            acc_sb = ppool.tile([128, D], FP, tag="acc_sb", name="acc_sb")
            for hf in range(NHALF):
                t0 = hf * 128
                moved = MOVED[hf]
                pe_grps = [
                    (gi, [k for k in grp if k not in moved])
                    for gi, grp in enumerate(GROUPS)
                ]
                pe_first = next(g[1][-1] for g in pe_grps if g[1])
                pe_last_gi, pe_last_grp = [g for g in pe_grps if g[1]][-1]
                first_moved = min(moved) if moved else None
                acc = psacc.tile([128, D], FP, tag="acc")
                cts = {}
                for gi, grp in enumerate(GROUPS):
                    for k in grp:
                        ct = gdpool.tile([128, 2 * D], BF, tag="ct")
                        cts[k] = ct
                        nc.gpsimd.indirect_dma_start(
                            out=ct[:],
                            out_offset=None,
                            in_=cat_d.ap(),
                            in_offset=bass.IndirectOffsetOnAxis(
                                ap=fi_all[hf][:, k : k + 1], axis=0
                            ),
                        )
                        # down-proj dot: hid[:,k] = sum(x_bf * ct[:, :D])
                        scr = spool.tile([128, D], BF, tag="scr")
                        nc.vector.tensor_tensor(
                            out=scr[:],
                            in0=ct[:, 0:D],
                            in1=x_bf[:, hf * D : (hf + 1) * D],
                            op=mybir.AluOpType.mult,
                        )
                        scr2 = spool.tile([128, D], BF, tag="scr2")
                        nc.scalar.activation(
                            scr2[:],
                            scr[:],
                            mybir.ActivationFunctionType.Copy,
                            accum_out=hid_all[hf][:, k : k + 1],
                        )
                    # h = silu(dot) for the whole group in one ACT op
                    nc.scalar.activation(
                        hsil_all[hf][:, grp[0] : grp[-1] + 1],
                        hid_all[hf][:, grp[0] : grp[-1] + 1],
                        mybir.ActivationFunctionType.Silu,
                    )
                    dgs = {}
                    for k in grp:
                        if k in moved:
                            # DVE up-proj: acc_sb (+)= ct_up * silu(dot) * relu(s)
                            if k == first_moved:
                                nc.vector.tensor_scalar(
                                    out=acc_sb[:],
                                    in0=cts[k][:, D : 2 * D],
                                    scalar1=hsil_all[hf][:, k : k + 1],
                                    scalar2=fsr_all[hf][:, k : k + 1],
                                    op0=mybir.AluOpType.mult,
                                    op1=mybir.AluOpType.mult,
                                )
                            else:
                                tmp = spool.tile([128, D], BF, tag="utmp")
                                nc.vector.tensor_scalar(
                                    out=tmp[:],
                                    in0=cts[k][:, D : 2 * D],
                                    scalar1=hsil_all[hf][:, k : k + 1],
                                    scalar2=fsr_all[hf][:, k : k + 1],
                                    op0=mybir.AluOpType.mult,
                                    op1=mybir.AluOpType.mult,
                                )
                                nc.vector.tensor_tensor(
                                    out=acc_sb[:],
                                    in0=acc_sb[:],
                                    in1=tmp[:],
                                    op=mybir.AluOpType.add,
                                )
                            continue
                        dg = dgpool.tile([128, 128], BF, tag="dg")
                        dgs[k] = dg
                        nc.vector.tensor_scalar(
                            out=dg[:],
                            in0=ident[:],
                            scalar1=hsil_all[hf][:, k : k + 1],
                            scalar2=fsr_all[hf][:, k : k + 1],
                            op0=mybir.AluOpType.mult,
                            op1=mybir.AluOpType.mult,
                        )
                    for k in reversed(grp):
                        if k in moved:
                            continue
                        for c in range(4):
                            nc.tensor.matmul(
                                acc[:, c * 512 : (c + 1) * 512],
                                lhsT=dgs[k][:],
                                rhs=cts[k][:, D + c * 512 : D + (c + 1) * 512],
                                start=(gi == 0 and k == pe_first),
                                stop=(gi == pe_last_gi and k == pe_last_grp[0]),
                            )
                ob = opool.tile([128, D], FP, tag="ob")
                if moved:
                    nc.vector.tensor_tensor(
                        out=ob[:], in0=acc[:], in1=acc_sb[:], op=mybir.AluOpType.add
                    )
                else:
                    nc.vector.tensor_copy(ob[:], acc[:])
                nc.sync.dma_start(out_d.ap()[t0 : t0 + 128, :], ob[:])

    return nc


_CACHED = {}


def kernel(x, Wq, keys, e_down, e_up):
    x = np.asarray(x, dtype=np.float32)
    Wq = np.asarray(Wq, dtype=np.float32)
    keys = np.asarray(keys, dtype=np.float32)
    e_down = np.asarray(e_down, dtype=np.float32)
    e_up = np.asarray(e_up, dtype=np.float32)

    if "nc" not in _CACHED:
        _CACHED["nc"] = build_program()
    nc = _CACHED["nc"]

    xf = x.reshape(B * T, D)
    keyst = np.ascontiguousarray(keys.transpose(2, 3, 0, 1)).reshape(2 * DK, H * NK)
    # keyst[sub*64+dk, h*NK + nk] = keys[h, nk, sub, dk]
    ident = np.eye(128, dtype=ml_dtypes.bfloat16)
    iota64 = np.tile(np.arange(64, dtype=np.float32), (128, 1))

    # combined bf16 expert table: row e = e_down[e] || e_up[e]
    ecat = np.empty((E, 2 * D), dtype=ml_dtypes.bfloat16)
    ecat[:, :D] = e_down.astype(ml_dtypes.bfloat16)
    ecat[:, D:] = e_up.astype(ml_dtypes.bfloat16)

    xbf = xf.astype(ml_dtypes.bfloat16)
    NDCH = D // 128
    # wq_r[m*128+p, c*128+o] = Wq[c*128+p, m*128+o]
    wq_r = np.ascontiguousarray(
        Wq.reshape(NDCH, 128, 4, 128).transpose(2, 1, 0, 3).reshape(512, NDCH * 128)
    )

    in_maps = []
    for c in range(N_CORES):
        xs = np.ascontiguousarray(xf[c * TOK_PER_CORE : (c + 1) * TOK_PER_CORE])
        xbf_c = xbf[c * TOK_PER_CORE : (c + 1) * TOK_PER_CORE]
        # xbf_r[p, hf*D+d] = xbf_c[hf*128+p, d]
        xbf_r = np.ascontiguousarray(
            xbf_c.reshape(NHALF, 128, D).transpose(1, 0, 2).reshape(128, NHALF * D)
        )
        # xt_r[p, ch*256+t] = xs[t, ch*128+p]
        xt_r = np.ascontiguousarray(
            xs.T.reshape(NDCH, 128, TOK_PER_CORE)
            .transpose(1, 0, 2)
            .reshape(128, NDCH * TOK_PER_CORE)
        )
        in_maps.append(
            {
                "xbf": xbf_r,
                "xt": xt_r,
                "wq": wq_r,
                "keyst": keyst,
                "ecat": ecat,
                "ident": ident,
                "iota64": iota64,
            }
        )

    res = run_bass_kernel_spmd(nc, in_maps, core_ids=list(range(N_CORES)))
    _CACHED["res"] = res
    out = np.concatenate([res.results[c]["out"] for c in range(N_CORES)], axis=0)
    return out.reshape(B, T, D)


# revision 55
# speedup vs baseline: 1.0936x; 1.0790x over previous
"""LlamaPEER MoE-routing kernel for 8 NeuronCores (TRN2, Bass/Tile).

Strategy: data-parallel over B*T (2048 tokens -> 256/core), expert tables
replicated. Host concatenates e_down|e_up into one bf16 table [E, 2D] so a
single indirect-DMA gather per (token-half, slot) fetches both rows.
Per core:
  1. qT = Wq^T @ x^T on PE (fp32 -- routing is precision-critical; xT
     precomputed on host). Interleaved per head so head-0 routing starts
     after 1/4 of the qT work.
  2. sims via PE (keysT precomputed on host), top-8 via DVE max/max_index,
     K x K cross combine + second top-8, index select via iota-mask trick.
  3. Per (h,k) slot: one indirect-DMA gather of the combined bf16 row
     (token-major: partition p = token p's row); down-proj dot via one
     fused DVE tensor_tensor_reduce against resident bf16 x tile; silu on
     ACT; dg = ident * silu(dot) * relu(score) in one DVE tensor_scalar;
     up-proj diag(h) @ up-half rows on PE, accumulated in PSUM over all
     32 slots; single copy-out per 128-token half.
"""

import numpy as np
import ml_dtypes

import concourse.bass as bass
import concourse.tile as tile
from concourse import mybir
from concourse.bass_utils import run_bass_kernel_spmd
from concourse.vector_clock import ScopedClock

N_CORES = 8
B, T, D = 2, 1024, 2048
H, K, DK = 4, 8, 64
E = 16384
NK = 128
TOK_PER_CORE = (B * T) // N_CORES  # 256
NHALF = TOK_PER_CORE // 128  # 2
NSLOT = H * K  # 32
FP = mybir.dt.float32
BF = mybir.dt.bfloat16
I32 = mybir.dt.int32
U32 = mybir.dt.uint32

# --- workaround: this walrus build allows only 1 sync-wait command on the
# final SP drain; split the tile-context drain into 1-wait drains.
_MAX_DRAIN_WAITS = 1


def _patched_drain_and_barrier(self, tick_clock, wait_clock):
    nc = self.nc
    drain_inst = nc.sync.drain()
    wait_clock.add_sem_waits(
        drain_inst.ins, ScopedClock({None: tick_clock.global_clock})
    )
    si = drain_inst.ins.sync_info
    if si is not None and len(si.on_wait) > _MAX_DRAIN_WAITS:
        waits = list(si.on_wait)
        upds = list(si.on_update)
        drain_inst.ins.sync_info = mybir.SyncInfo(
            on_wait=waits[:_MAX_DRAIN_WAITS], on_update=[]
        )
        rest = waits[_MAX_DRAIN_WAITS:]
        while rest:
            extra = nc.sync.drain()
            extra.ins.sync_info = mybir.SyncInfo(
                on_wait=rest[:_MAX_DRAIN_WAITS],
                on_update=upds if len(rest) <= _MAX_DRAIN_WAITS else [],
            )
            rest = rest[_MAX_DRAIN_WAITS:]
    nc.all_engine_barrier()
    popped = nc._tile_sem_poison_stack.pop()
    assert popped is self._sem_poison
    all_sems = list(self.sems.allocated().values())
    for i in range(0, len(all_sems), 16):
        nc.clear_and_free_semaphores(all_sems[i : i + 16])
    nc.all_engine_barrier()


tile.TileContext._drain_and_barrier = _patched_drain_and_barrier

_orig_lower_ordered = tile.TileContext._lower_ordered_insts


def _patched_lower_ordered(self, postordered_blocks):
    # this walrus build supports only one sync-wait command per instruction:
    # hoist extra waits onto same-engine NoOps placed just before.
    for bb_name, insts in postordered_blocks.items():
        new = []
        for inst in insts:
            si = getattr(inst, "sync_info", None)
            eng = getattr(inst, "engine", None)
            if si is not None and eng is not None and len(si.on_wait) > 1:
                waits = list(si.on_wait)
                for w in waits[:-1]:
                    nop = mybir.InstNoOp(
                        name=self.nc.get_next_instruction_name(),
                        sync_info=mybir.SyncInfo(on_wait=[w], on_update=[]),
                        bass_nofuse=True,
                        engine=eng,
                    )
                    new.append(nop)
                inst.sync_info = mybir.SyncInfo(
                    on_wait=[waits[-1]], on_update=list(si.on_update)
                )
            new.append(inst)
        insts[:] = new
    return _orig_lower_ordered(self, postordered_blocks)


tile.TileContext._lower_ordered_insts = _patched_lower_ordered


def _re(ap, dims):
    """Return ap with its free-axis access pattern replaced by `dims`
    (list of [step, count]); keeps the partition dim."""
    return ap.__replace__(ap=[list(ap.ap)[0]] + [list(d) for d in dims])


def build_program():
    nc = bass.Bass("TRN2", target_bir_lowering=False, debug=False)

    # host pre-laid-out for contiguous per-partition loads:
    # xbf[p, hf*D+d], xt[p, c*256+t], wq[m*128+p, c*128+o]
    xbf_d = nc.dram_tensor("xbf", [128, NHALF * D], BF, kind="ExternalInput")
    xt_d = nc.dram_tensor("xt", [128, (D // 128) * TOK_PER_CORE], FP, kind="ExternalInput")
    wq_d = nc.dram_tensor("wq", [4 * 128, (D // 128) * 128], FP, kind="ExternalInput")
    kt_d = nc.dram_tensor("keyst", [2 * DK, H * NK], FP, kind="ExternalInput")
    cat_d = nc.dram_tensor("ecat", [E, 2 * D], BF, kind="ExternalInput")
    id_d = nc.dram_tensor("ident", [128, 128], BF, kind="ExternalInput")
    io_d = nc.dram_tensor("iota64", [128, 64], FP, kind="ExternalInput")
    out_d = nc.dram_tensor("out", [TOK_PER_CORE, D], FP, kind="ExternalOutput")

    NDCH = D // 128  # 16 d-chunks

    with tile.TileContext(nc) as tc:
        with (
            tc.tile_pool(name="const", bufs=1) as cpool,
            tc.tile_pool(name="mats", bufs=1) as mpool,
            tc.tile_pool(name="route", bufs=4) as rpool,
            tc.tile_pool(name="persist", bufs=1) as ppool,
            tc.tile_pool(name="gd", bufs=11) as gdpool,
            tc.tile_pool(name="scr", bufs=2) as spool,
            tc.tile_pool(name="dg", bufs=10) as dgpool,
            tc.tile_pool(name="ob", bufs=1) as opool,
            tc.tile_pool(name="psq", bufs=4, space="PSUM") as psq,
            tc.tile_pool(name="psacc", bufs=1, space="PSUM") as psacc,
        ):
            # PE warm-up: the HAM throttle needs ~3.4us of continuous
            # activity before the PE runs at full clock. While the input
            # DMAs stream (~17us), keep the PE busy with dummy matmuls on
            # a memset tile so qT starts warm instead of at half clock.
            warm = cpool.tile([128, 256], BF, tag="warm", name="warm")
            nc.vector.memset(warm[:], 0.0)
            pwarm = psq.tile([128, 256], FP, tag="psq")
            for _ in range(52):
                nc.tensor.matmul(
                    pwarm[:],
                    lhsT=warm[:, 0:128],
                    rhs=warm[:],
                    start=True,
                    stop=True,
                )

            # load order tuned for pipeline startup: xt + wq m-group 0
            # first (qT m0 can start ~10us in), tiny/late-needed after.
            # all loads are contiguous per partition (host pre-laid-out).
            half_cols = (NDCH // 2) * TOK_PER_CORE
            xt_a = mpool.tile([128, half_cols], FP, tag="xt_a", name="xt_a")
            xt_b = mpool.tile([128, half_cols], FP, tag="xt_b", name="xt_b")

            def xt_chunk(c, a=0, b=TOK_PER_CORE):
                # qT chunk c's rhs: chunks 0-7 live in xt_a, 8-15 in xt_b
                t = xt_a if c < NDCH // 2 else xt_b
                c2 = c % (NDCH // 2)
                return t[:, c2 * TOK_PER_CORE + a : c2 * TOK_PER_CORE + b]

            nc.sync.dma_start(xt_a[:], xt_d.ap()[:, :half_cols])
            wqm = []
            for m in range(4):
                wt = mpool.tile([128, NDCH * 128], FP, tag=f"wqm{m}", name=f"wqm{m}")
                if m == 0:
                    # scalar HWDGE ring: loads in parallel with xt_a on the
                    # sync ring -- both are on the first-gather critical path
                    nc.scalar.dma_start(wt[:], wq_d.ap()[0:128, :])
                else:
                    nc.sync.dma_start(wt[:], wq_d.ap()[m * 128 : (m + 1) * 128, :])
                wqm.append(wt)
                if m == 0:
                    nc.sync.dma_start(xt_b[:], xt_d.ap()[:, half_cols:])
                    kt_sb = mpool.tile([2 * DK, H * NK], FP)
                    nc.sync.dma_start(kt_sb[:], kt_d.ap())
                    # x rows in bf16 for the down-proj dot (both halves)
                    x_bf = mpool.tile([128, NHALF * D], BF)
                    nc.sync.dma_start(x_bf[:], xbf_d.ap())
                    ident = cpool.tile([128, 128], BF)
                    nc.sync.dma_start(ident[:], id_d.ap())
                    iota = cpool.tile([128, 64], FP)
                    nc.sync.dma_start(iota[:], io_d.ap())

            qt_sb = ppool.tile([128, 4 * TOK_PER_CORE], FP)
            fi_all = [
                ppool.tile([128, NSLOT], I32, tag=f"fi{hf}", name=f"fi{hf}")
                for hf in range(NHALF)
            ]
            fsr_all = [
                ppool.tile([128, NSLOT], FP, tag=f"fsr{hf}", name=f"fsr{hf}")
                for hf in range(NHALF)
            ]
            hid_all = [
                ppool.tile([128, NSLOT], FP, tag=f"hid{hf}", name=f"hid{hf}")
                for hf in range(NHALF)
            ]
            hsil_all = [
                ppool.tile([128, NSLOT], FP, tag=f"hsil{hf}", name=f"hsil{hf}")
                for hf in range(NHALF)
            ]

            # ---- phase 1: qT + routing. PE emission order is
            # m0, m1, rt0, m2, rt1, m3, rt2, rt3 so the PE never waits on
            # the PSUM->SBUF copy of the m-group it needs for sims (the
            # copy runs during the next m-group's matmuls), keeping the
            # tensor engine continuously busy so its p-state ramps. ----
            def emit_qt(h, tr=(0, TOK_PER_CORE)):
                a, b = tr
                pq = psq.tile([128, b - a], FP, tag="psq")
                for c in range(NDCH):
                    nc.tensor.matmul(
                        pq[:],
                        lhsT=wqm[h][:, c * 128 : (c + 1) * 128],
                        rhs=xt_chunk(c, a, b),
                        start=(c == 0),
                        stop=(c == NDCH - 1),
                    )
                nc.vector.tensor_copy(
                    qt_sb[:, h * TOK_PER_CORE + a : h * TOK_PER_CORE + b], pq[:]
                )

            def emit_routing(h, halves=(0, 1)):
                for hf in halves:
                    t0 = hf * 128
                    ss = []
                    ii = []
                    for sub in range(2):
                        ps = psq.tile([128, NK], FP, tag="psq")
                        nc.tensor.matmul(
                            ps[:],
                            lhsT=qt_sb[
                                sub * 64 : (sub + 1) * 64,
                                h * TOK_PER_CORE + t0 : h * TOK_PER_CORE + t0 + 128,
                            ],
                            rhs=kt_sb[sub * 64 : (sub + 1) * 64, h * NK : (h + 1) * NK],
                            start=True,
                            stop=True,
                        )
                        s = rpool.tile([128, 8], FP, tag="s")
                        nc.vector.max(s[:], ps[:])
                        idx = rpool.tile([128, 8], U32, tag="idx")
                        nc.vector.max_index(idx[:], s[:], ps[:])
                        idf = rpool.tile([128, 8], FP, tag="idf")
                        nc.vector.tensor_copy(idf[:], idx[:])
                        ss.append(s)
                        ii.append(idf)
                    # cross combine: [128, 8(k1), 8(k2)]
                    alls = rpool.tile([128, 64], FP, tag="alls")
                    a3 = _re(alls[:], [[8, 8], [1, 8]])
                    nc.vector.tensor_tensor(
                        out=a3,
                        in0=_re(ss[0][:], [[1, 8], [0, 8]]),
                        in1=_re(ss[1][:], [[0, 8], [1, 8]]),
                        op=mybir.AluOpType.add,
                    )
                    alli = rpool.tile([128, 64], FP, tag="alli")
                    ai3 = _re(alli[:], [[8, 8], [1, 8]])
                    nc.vector.tensor_scalar(
                        out=ai3,
                        in0=_re(ii[0][:], [[1, 8], [0, 8]]),
                        scalar1=float(NK),
                        scalar2=None,
                        op0=mybir.AluOpType.mult,
                    )
                    nc.vector.tensor_tensor(
                        out=ai3,
                        in0=ai3,
                        in1=_re(ii[1][:], [[0, 8], [1, 8]]),
                        op=mybir.AluOpType.add,
                    )
                    fs = rpool.tile([128, 8], FP, tag="fs")
                    nc.vector.max(fs[:], alls[:])
                    pk = rpool.tile([128, 8], U32, tag="pk")
                    nc.vector.max_index(pk[:], fs[:], alls[:])
                    pkf = rpool.tile([128, 8], FP, tag="pkf")
                    nc.vector.tensor_copy(pkf[:], pk[:])
                    # mask[p, j, n] = (pk[p,j] == iota[p,n])
                    mask = rpool.tile([128, 512], FP, tag="mask")
                    m3 = _re(mask[:], [[64, 8], [1, 64]])
                    nc.vector.tensor_tensor(
                        out=m3,
                        in0=_re(pkf[:], [[1, 8], [0, 64]]),
                        in1=_re(iota[:], [[0, 8], [1, 64]]),
                        op=mybir.AluOpType.is_equal,
                    )
                    nc.vector.tensor_tensor(
                        out=m3,
                        in0=m3,
                        in1=_re(alli[:], [[0, 8], [1, 64]]),
                        op=mybir.AluOpType.mult,
                    )
                    fif = rpool.tile([128, 8], FP, tag="fif")
                    nc.vector.tensor_reduce(
                        fif[:],
                        m3,
                        axis=mybir.AxisListType.X,
                        op=mybir.AluOpType.add,
                    )
                    nc.vector.tensor_copy(fi_all[hf][:, h * 8 : (h + 1) * 8], fif[:])
                    nc.vector.tensor_scalar_max(
                        fsr_all[hf][:, h * 8 : (h + 1) * 8], fs[:], 0.0
                    )

            # head 0's qT is split by token-half so the first half's
            # routing (and therefore the first gathers) starts after only
            # 16 half-width matmuls instead of the full m-group.
            emit_qt(0, tr=(0, 128))
            emit_routing(0, halves=(0,))
            emit_qt(0, tr=(128, 256))
            emit_routing(0, halves=(1,))
            emit_qt(1)
            emit_routing(1)
            emit_qt(2)
            emit_routing(2)
            emit_qt(3)
            emit_routing(3)

            # ---- phase 2: per-slot gather + fused down/up projection.
            # Slots processed in groups of GRP: gathers/dots stream in slot
            # order, but the up-proj matmuls are emitted in REVERSE slot
            # order within each group so the PE idles once per group and
            # then bursts 4*GRP matmuls back-to-back (keeps the HAM
            # throttle/p-state at full clock instead of oscillating
            # per-slot). silu is batched per group. ----
            GROUPS = [list(range(g, g + 4)) for g in range(0, 28, 4)] + [
                [28, 29],
                [30, 31],
            ]
            # In the second half the PE has built up a queue backlog
            # (prologue qT/sims overlaps the first half's stream), so it
            # trails the gather stream at the tail. Offload the up-proj of
            # these hf1 slots to the DVE (scale + add into an SBUF
            # accumulator, merged with the PSUM accumulator at the end).
            MOVED = {0: set(), 1: {3, 7, 11, 15, 19, 23, 27}}
            acc_sb = ppool.tile([128, D], FP, tag="acc_sb", name="acc_sb")
            for hf in range(NHALF):
                t0 = hf * 128
                moved = MOVED[hf]
                pe_grps = [
                    (gi, [k for k in grp if k not in moved])
                    for gi, grp in enumerate(GROUPS)
                ]
                pe_first = next(g[1][-1] for g in pe_grps if g[1])
                pe_last_gi, pe_last_grp = [g for g in pe_grps if g[1]][-1]
                first_moved = min(moved) if moved else None
                acc = psacc.tile([128, D], FP, tag="acc")
                cts = {}
                for gi, grp in enumerate(GROUPS):
                    for k in grp:
                        ct = gdpool.tile([128, 2 * D], BF, tag="ct")
                        cts[k] = ct
                        nc.gpsimd.indirect_dma_start(
                            out=ct[:],
                            out_offset=None,
                            in_=cat_d.ap(),
                            in_offset=bass.IndirectOffsetOnAxis(
                                ap=fi_all[hf][:, k : k + 1], axis=0
                            ),
                        )
                        # down-proj dot: hid[:,k] = sum(x_bf * ct[:, :D])
                        scr = spool.tile([128, D], BF, tag="scr")
                        nc.vector.tensor_tensor(
                            out=scr[:],
                            in0=ct[:, 0:D],
                            in1=x_bf[:, hf * D : (hf + 1) * D],
                            op=mybir.AluOpType.mult,
                        )
                        scr2 = spool.tile([128, D], BF, tag="scr2")
                        nc.scalar.activation(
                            scr2[:],
                            scr[:],
                            mybir.ActivationFunctionType.Copy,
                            accum_out=hid_all[hf][:, k : k + 1],
                        )
                    # h = silu(dot) for the whole group in one ACT op
                    nc.scalar.activation(
                        hsil_all[hf][:, grp[0] : grp[-1] + 1],
                        hid_all[hf][:, grp[0] : grp[-1] + 1],
                        mybir.ActivationFunctionType.Silu,
                    )
                    dgs = {}
                    for k in grp:
                        if k in moved:
                            # DVE up-proj: acc_sb (+)= ct_up * silu(dot) * relu(s)
                            if k == first_moved:
                                nc.vector.tensor_scalar(
                                    out=acc_sb[:],
                                    in0=cts[k][:, D : 2 * D],
                                    scalar1=hsil_all[hf][:, k : k + 1],
                                    scalar2=fsr_all[hf][:, k : k + 1],
                                    op0=mybir.AluOpType.mult,
                                    op1=mybir.AluOpType.mult,
                                )
                            else:
                                tmp = spool.tile([128, D], BF, tag="utmp")
                                nc.vector.tensor_scalar(
                                    out=tmp[:],
                                    in0=cts[k][:, D : 2 * D],
                                    scalar1=hsil_all[hf][:, k : k + 1],
                                    scalar2=fsr_all[hf][:, k : k + 1],
                                    op0=mybir.AluOpType.mult,
                                    op1=mybir.AluOpType.mult,
                                )
                                nc.vector.tensor_tensor(
                                    out=acc_sb[:],
                                    in0=acc_sb[:],
                                    in1=tmp[:],
                                    op=mybir.AluOpType.add,
                                )
                            continue
                        dg = dgpool.tile([128, 128], BF, tag="dg")
                        dgs[k] = dg
                        nc.vector.tensor_scalar(
                            out=dg[:],
                            in0=ident[:],
                            scalar1=hsil_all[hf][:, k : k + 1],
                            scalar2=fsr_all[hf][:, k : k + 1],
                            op0=mybir.AluOpType.mult,
                            op1=mybir.AluOpType.mult,
                        )
                    for k in reversed(grp):
                        if k in moved:
                            continue
                        for c in range(4):
                            nc.tensor.matmul(
                                acc[:, c * 512 : (c + 1) * 512],
                                lhsT=dgs[k][:],
                                rhs=cts[k][:, D + c * 512 : D + (c + 1) * 512],
                                start=(gi == 0 and k == pe_first),
                                stop=(gi == pe_last_gi and k == pe_last_grp[0]),
                            )
                # merge + store in 512-col chunks so the first chunk's DMA
                # overlaps the remaining chunks' merges at the tail
                ob = opool.tile([128, D], FP, tag="ob")
                for c in range(4):
                    cs = slice(c * 512, (c + 1) * 512)
                    if moved:
                        nc.vector.tensor_tensor(
                            out=ob[:, cs],
                            in0=acc[:, cs],
                            in1=acc_sb[:, cs],
                            op=mybir.AluOpType.add,
                        )
                    else:
                        nc.vector.tensor_copy(ob[:, cs], acc[:, cs])
                    nc.sync.dma_start(
                        out_d.ap()[t0 : t0 + 128, c * 512 : (c + 1) * 512],
                        ob[:, cs],
                    )

    return nc


_CACHED = {}


def kernel(x, Wq, keys, e_down, e_up):
    x = np.asarray(x, dtype=np.float32)
    Wq = np.asarray(Wq, dtype=np.float32)
    keys = np.asarray(keys, dtype=np.float32)
    e_down = np.asarray(e_down, dtype=np.float32)
    e_up = np.asarray(e_up, dtype=np.float32)

    if "nc" not in _CACHED:
        _CACHED["nc"] = build_program()
    nc = _CACHED["nc"]

    xf = x.reshape(B * T, D)
    keyst = np.ascontiguousarray(keys.transpose(2, 3, 0, 1)).reshape(2 * DK, H * NK)
    # keyst[sub*64+dk, h*NK + nk] = keys[h, nk, sub, dk]
    ident = np.eye(128, dtype=ml_dtypes.bfloat16)
    iota64 = np.tile(np.arange(64, dtype=np.float32), (128, 1))

    # combined bf16 expert table: row e = e_down[e] || e_up[e]
    ecat = np.empty((E, 2 * D), dtype=ml_dtypes.bfloat16)
    ecat[:, :D] = e_down.astype(ml_dtypes.bfloat16)
    ecat[:, D:] = e_up.astype(ml_dtypes.bfloat16)

    xbf = xf.astype(ml_dtypes.bfloat16)
    NDCH = D // 128
    # wq_r[m*128+p, c*128+o] = Wq[c*128+p, m*128+o]
    wq_r = np.ascontiguousarray(
        Wq.reshape(NDCH, 128, 4, 128).transpose(2, 1, 0, 3).reshape(512, NDCH * 128)
    )

    in_maps = []
    for c in range(N_CORES):
        xs = np.ascontiguousarray(xf[c * TOK_PER_CORE : (c + 1) * TOK_PER_CORE])
        xbf_c = xbf[c * TOK_PER_CORE : (c + 1) * TOK_PER_CORE]
        # xbf_r[p, hf*D+d] = xbf_c[hf*128+p, d]
        xbf_r = np.ascontiguousarray(
            xbf_c.reshape(NHALF, 128, D).transpose(1, 0, 2).reshape(128, NHALF * D)
        )
        # xt_r[p, ch*256+t] = xs[t, ch*128+p]
        xt_r = np.ascontiguousarray(
            xs.T.reshape(NDCH, 128, TOK_PER_CORE)
            .transpose(1, 0, 2)
            .reshape(128, NDCH * TOK_PER_CORE)
        )
        in_maps.append(
            {
                "xbf": xbf_r,
                "xt": xt_r,
                "wq": wq_r,
                "keyst": keyst,
                "ecat": ecat,
                "ident": ident,
                "iota64": iota64,
            }
        )

    res = run_bass_kernel_spmd(nc, in_maps, core_ids=list(range(N_CORES)))
    _CACHED["res"] = res
    out = np.concatenate([res.results[c]["out"] for c in range(N_CORES)], axis=0)
    return out.reshape(B, T, D)
